# revision 24
# baseline (speedup 1.0000x reference)
"""Distributed causal multi-head attention for 8 TRN2 NeuronCores (v2).

Problem: B=2, T=2048, D=1024, H=16 heads (hd=64), f32 in/out.

Sharding: core i handles batch b=i//4 and head-group g=i%4 (4 heads).
Wq/Wk/Wv column-sharded ([1024, 256] per core), Wo row-sharded
([256, 1024] per core).  Each core computes a partial output projection
for its 4 heads over the full sequence; the host sums the 4 partials
per batch (the unshard step replaces the all-reduce).  Weights and
activations are pre-cast to bf16 on the host; x is laid out transposed
(xT = x^T).  Output partials are shipped bf16 and summed f32 on host.

v2 dataflow changes vs v1:
  - softmax normalize applied directly on the AV output (O^T layout,
    dh on partitions) via: den-row cast -> K=1 ones-matmul broadcast
    (den replicated over 64 partitions in PSUM) -> reciprocal_approx
    -> one tensor_tensor multiply writing attnT in place.  This deletes
    all 96 PE transposes of v1 (4 per pair + 2 per q-tile).
  - odd heads (attnT partitions 64-127) land via a cheap SBUF->SBUF
    DMA partition shift (DVE lanes are partition-locked).
  - QKV projections stream as 512-column wavefront thunks used as PE
    filler: wave c feeds q-slab c, emitted one slab ahead, so attention
    starts ~3us into the kernel and the PE never starves on input DMA.
  - out projection results are cast bf16 and DMA'd per q-tile (halves
    output traffic; host sums partials in f32).
  - exp table-load is prefetched with a dummy activation at t=0.
"""

import numpy as np
import ml_dtypes

import concourse.bass as bass
import concourse.mybir as mybir
import concourse.tile as tile
from concourse import bacc
from concourse.bass_utils import run_bass_kernel_spmd

F32 = mybir.dt.float32
BF16 = mybir.dt.bfloat16
AF = mybir.ActivationFunctionType
MULT = mybir.AluOpType.mult

T = 2048  # sequence length
D = 1024  # embed dim
NH = 4  # heads per core
HD = 64  # head dim
DH = NH * HD  # 256, sharded d per core
TT = T // 128  # 16 t tiles
DT = D // 128  # 8 embed tiles
NSLAB = 4  # q slabs of 512
SCALE = 1.0 / np.sqrt(HD)

_NC_CACHE = None


def build():
    nc = bacc.Bacc(None, target_bir_lowering=False, debug=False)

    # inputs are shipped as ready-to-DMA SBUF images (see make_in_maps):
    # xT_img[p, c*4096 + dt*512 + j] = x[c*512+j, dt*128+p]  (chunk-major)
    # wq/wk m-major [p, m*1024 + dt*128 + c]; wv dt-major [p, dt*256 + c];
    # wo i-major [p, i*1024 + c]
    xT_img = nc.declare_dram_parameter("xT", [128, NSLAB * DT * 512], BF16, isOutput=False)
    wq = nc.declare_dram_parameter("Wq", [128, 2 * DT * 128], BF16, isOutput=False)
    wk = nc.declare_dram_parameter("Wk", [128, 2 * DT * 128], BF16, isOutput=False)
    wv = nc.declare_dram_parameter("Wv", [128, DT * DH], BF16, isOutput=False)
    wo = nc.declare_dram_parameter("Wo", [128, 2 * D], BF16, isOutput=False)
    out = nc.declare_dram_parameter("out", [T, D], BF16, isOutput=True)

    with tile.TileContext(nc) as tc:
        with (
            tc.tile_pool(name="persist", bufs=1) as persist,
            tc.tile_pool(name="pt", bufs=2) as pt_pool,
            tc.tile_pool(name="den", bufs=2) as den_pool,
            tc.tile_pool(name="rc", bufs=2) as rc_pool,
            tc.tile_pool(name="stg", bufs=2) as stg_pool,
            tc.tile_pool(name="osb", bufs=2) as osb_pool,
            tc.tile_pool(name="ps_big", bufs=2, space="PSUM") as ps_big,
            tc.tile_pool(name="ps_fill", bufs=2, space="PSUM") as ps_fill,
            tc.tile_pool(name="ps_av", bufs=2, space="PSUM") as ps_av,
        ):
            def P(shape, dtype, name):
                return persist.tile(shape, dtype, name=name, tag=name)

            ones_b = P([128, 64], BF16, "ones_b")
            junk = P([128, 16], F32, "junk")
            jout = P([128, 16], F32, "jout")
            jnk_b = P([128, 512], BF16, "jnk_b")

            wq_bf = P([128, DT * DH], BF16, "wq_bf")
            wk_bf = P([128, DT * DH], BF16, "wk_bf")
            wv_bf = P([128, DT * DH], BF16, "wv_bf")
            wo_bf = P([128, 2 * D], BF16, "wo_bf")
            xT = P([128, DT * T], BF16, "xT")
            QT = P([128, 2 * T], BF16, "QT")
            KT = P([128, 2 * T], BF16, "KT")
            vbuf = P([128, TT * NH * 65], BF16, "vbuf")
            attnT = P([128, 2 * T], BF16, "attnT")

            # memsets first (gpsimd, ~0.4us) so the HAM warm-up dummies
            # can start at ~7us while the input DMAs land
            nc.gpsimd.memset(ones_b[:], 1.0)
            nc.gpsimd.memset(junk[:], 0.0)
            nc.gpsimd.memset(jnk_b[0:1, :], 1.0)
            # exp table prefetch: overlaps the ~2.7us ACT_TABLE_LOAD with
            # the input DMAs instead of paying it at the first real score
            nc.scalar.activation(out=jout[:], in_=junk[:], func=AF.Exp, scale=1.0)
            # ---- input DMAs: issued first, pre-arranged images, 8-32KB
            # contiguous lines, split across sync/gpsimd/scalar queues ----
            xT3 = xT.rearrange("p (dt t) -> p dt t", dt=DT)

            def xi3(c, d0, d1):
                return xT_img[:, c * 4096 + d0 * 512 : c * 4096 + d1 * 512].rearrange(
                    "p (dt t) -> p dt t", dt=d1 - d0
                )

            def xc_dma(eng, c, d0, d1):
                eng.dma_start(
                    out=xT3[:, d0:d1, c * 512 : (c + 1) * 512], in_=xi3(c, d0, d1)
                )

            # first wave = exactly what the first thunks need (wq_m0 +
            # xT chunk 0, 1.25MB): the three rings share the 16 DMA
            # engines, so nothing else competes until these are in flight
            nc.sync.dma_start(out=wq_bf[:, 0:1024], in_=wq[:, 0:1024])
            xc_dma(nc.scalar, 0, 0, 4)
            xc_dma(nc.gpsimd, 0, 4, 8)
            nc.sync.dma_start(out=wq_bf[:, 1024:2048], in_=wq[:, 1024:2048])
            nc.scalar.dma_start(out=wk_bf[:, 0:1024], in_=wk[:, 0:1024])
            nc.gpsimd.dma_start(out=wk_bf[:, 1024:2048], in_=wk[:, 1024:2048])
            nc.sync.dma_start(out=wv_bf[:], in_=wv[:])
            xc_dma(nc.scalar, 1, 0, 4)
            xc_dma(nc.gpsimd, 1, 4, 8)
            nc.sync.dma_start(out=wo_bf[:], in_=wo[:])
            xc_dma(nc.scalar, 2, 0, 4)
            xc_dma(nc.gpsimd, 2, 4, 8)
            xc_dma(nc.sync, 3, 0, 4)
            xc_dma(nc.gpsimd, 3, 4, 8)

            # HAM warm-up: ~8 dummy matmuls (~4us cold) run during the
            # DMA wait so the first real matmuls start near 2.4GHz
            warm_ps = ps_fill.tile([128, 512], F32, name="warm", tag="fill")
            for _ in range(8):
                nc.tensor.matmul(
                    warm_ps[0:64, 0:512],
                    lhsT=ones_b[0:1, 0:64],
                    rhs=jnk_b[0:1, 0:512],
                    start=True,
                    stop=True,
                )
            vb3 = vbuf.rearrange("p (t c) -> p t c", c=65)
            nc.gpsimd.memset(vb3[:, :, 64:65], 1.0)
            vb4 = vbuf.rearrange("p (n c) -> p n c", c=65)

            # ---- projection wavefront thunks (PE filler) ----
            def qk_thunks(c):
                th = []
                for w_bf, outT in ((wq_bf, QT), (wk_bf, KT)):
                    for m in range(2):
                        def go(w_bf=w_bf, outT=outT, m=m, c=c):
                            ps = ps_fill.tile([128, 512], F32, name="qk", tag="fill")
                            for dt_ in range(DT):
                                nc.tensor.matmul(
                                    ps[:],
                                    lhsT=w_bf[
                                        :,
                                        m * 1024 + dt_ * 128 : m * 1024 + (dt_ + 1) * 128,
                                    ],
                                    rhs=xT[
                                        :, dt_ * T + c * 512 : dt_ * T + (c + 1) * 512
                                    ],
                                    start=(dt_ == 0),
                                    stop=(dt_ == DT - 1),
                                )
                            nc.vector.tensor_copy(
                                outT[:, m * T + c * 512 : m * T + (c + 1) * 512],
                                ps[:],
                            )

                        th.append(go)
                return th

            def v_thunks(tts):
                th = []
                for tt in tts:
                    def go(tt=tt):
                        ps = ps_fill.tile([128, 256], F32, name="vp", tag="fill")
                        for dt_ in range(DT):
                            nc.tensor.matmul(
                                ps[:],
                                lhsT=xT[
                                    :, dt_ * T + tt * 128 : dt_ * T + (tt + 1) * 128
                                ],
                                rhs=wv_bf[:, dt_ * DH : (dt_ + 1) * DH],
                                start=(dt_ == 0),
                                stop=(dt_ == DT - 1),
                            )
                        nc.vector.tensor_copy(
                            vb4[:, tt * NH : (tt + 1) * NH, 0:64],
                            ps.rearrange("p (n c) -> p n c", n=NH),
                        )

                    th.append(go)
                return th

            # ---- scores ----
            def pt_layout(s):
                """Compact per-pair PT layout: col base and q-offset per kt."""
                base, off, b = {}, {}, 0
                for kt in range(4 * (s + 1)):
                    j = kt - 4 * s
                    o = 128 * j if j > 0 else 0
                    base[kt], off[kt] = b, o
                    b += 512 - o
                return base, off

            def scores_chunks(s, h, pt):
                m, r0 = h // 2, (h % 2) * 64
                base, _ = pt_layout(s)

                def off_diag(kt):
                    def go():
                        ps = ps_big.tile([128, 1024], F32, name="psst")
                        for u in range(2):
                            nc.tensor.matmul(
                                ps[:, u * 512 : (u + 1) * 512],
                                lhsT=KT[
                                    r0 : r0 + 64,
                                    m * T + (kt + u) * 128 : m * T + (kt + u + 1) * 128,
                                ],
                                rhs=QT[
                                    r0 : r0 + 64,
                                    m * T + s * 512 : m * T + (s + 1) * 512,
                                ],
                                start=True,
                                stop=True,
                            )
                        nc.scalar.activation(
                            out=pt[:, base[kt] : base[kt] + 1024],
                            in_=ps[:],
                            func=AF.Exp,
                            scale=float(SCALE),
                        )

                    return go

                def diag2(j0):
                    widths = [512 - 128 * j0, 512 - 128 * (j0 + 1)]
                    wtot = sum(widths)

                    def go():
                        ps = ps_big.tile([128, 1024], F32, name="psst")
                        o = 0
                        for u, w in enumerate(widths):
                            j = j0 + u
                            kt = 4 * s + j
                            nc.tensor.matmul(
                                ps[:, o : o + w],
                                lhsT=KT[
                                    r0 : r0 + 64,
                                    m * T + kt * 128 : m * T + (kt + 1) * 128,
                                ],
                                rhs=QT[
                                    r0 : r0 + 64,
                                    m * T + s * 512 + 128 * j : m * T + (s + 1) * 512,
                                ],
                                start=True,
                                stop=True,
                            )
                            o += w
                        kt0 = 4 * s + j0
                        nc.scalar.activation(
                            out=pt[:, base[kt0] : base[kt0] + wtot],
                            in_=ps[:, 0:wtot],
                            func=AF.Exp,
                            scale=float(SCALE),
                        )
                        for u in range(2):
                            kt = 4 * s + j0 + u
                            nc.gpsimd.affine_select(
                                out=pt[:, base[kt] : base[kt] + 128],
                                in_=pt[:, base[kt] : base[kt] + 128],
                                pattern=[[1, 128]],
                                compare_op=mybir.AluOpType.is_ge,
                                fill=0.0,
                                base=0,
                                channel_multiplier=-1,
                            )

                    return go

                return [off_diag(2 * u) for u in range(2 * s)] + [diag2(0), diag2(2)]

            # ---- AV + normalize ----
            def av_ops_ab(s, h, pt, ref):
                """AV accumulation split: A = off-diag k-tiles (exp long
                done), B = the 4 diagonal k-tiles (gated on last exps)."""
                base, off = pt_layout(s)
                nk = 4 * (s + 1)

                def av_a():
                    avb = ps_av.tile([128, 512], F32, name="psav", tag="psav")
                    ref["avb"] = avb
                    for kt in range(4 * s):
                        nc.tensor.matmul(
                            avb[0:65, 0:512],
                            lhsT=vb4[:, kt * NH + h, :],
                            rhs=pt[:, base[kt] : base[kt] + 512],
                            start=(kt == 0),
                            stop=False,
                        )

                def av_b():
                    avb = ref["avb"]
                    for kt in range(4 * s, nk):
                        o = off[kt]
                        nc.tensor.matmul(
                            avb[0:65, o:512],
                            lhsT=vb4[:, kt * NH + h, :],
                            rhs=pt[:, base[kt] : base[kt] + 512 - o],
                            start=(kt == 0),
                            stop=(kt == nk - 1),
                        )

                return av_a, av_b

            def norm_ops(s, h, ref, q0, q1):
                """den row -> bf16 -> K=1 ones-matmul broadcast -> recip
                -> tensor_tensor multiply -> attnT (DMA shift for odd h)."""
                i_c = h // 2
                c0 = i_c * T + s * 512
                odd = h % 2 == 1
                st = {}

                def d1():
                    den = den_pool.tile([128, 512], BF16, name="den")
                    st["den"] = den
                    nc.vector.tensor_copy(
                        den[64:65, q0:q1], ref["avb"][64:65, q0:q1]
                    )

                def m1():
                    denb = ps_fill.tile([128, 512], F32, name="denb", tag="fill")
                    st["denb"] = denb
                    nc.tensor.matmul(
                        denb[0:64, q0:q1],
                        lhsT=ones_b[64:65, 0:64],
                        rhs=st["den"][64:65, q0:q1],
                        start=True,
                        stop=True,
                    )

                def d2():
                    rc = rc_pool.tile([128, 512], F32, name="rc")
                    st["rc"] = rc
                    nc.vector.reciprocal_approx_fast(
                        rc[0:64, q0:q1], st["denb"][0:64, q0:q1]
                    )

                def d3():
                    if odd:
                        stg = stg_pool.tile([128, 512], BF16, name="stg")
                        st["stg"] = stg
                        dst = stg[0:64, q0:q1]
                    else:
                        dst = attnT[0:64, c0 + q0 : c0 + q1]
                    nc.vector.tensor_tensor(
                        out=dst,
                        in0=ref["avb"][0:64, q0:q1],
                        in1=st["rc"][0:64, q0:q1],
                        op=MULT,
                    )

                def d4():
                    nc.gpsimd.dma_start(
                        out=attnT[64:128, c0 + q0 : c0 + q1],
                        in_=st["stg"][0:64, q0:q1],
                    )

                ops = [d1, m1, d2, d3]
                if odd:
                    ops.append(d4)
                return ops

            # ---- out projection epilogue.  tail=True (final slab): ec0
            # cast moves to the now-idle scalar engine and each 512-col
            # half DMAs out as soon as its cast lands ----
            def epilogue_ops(s, tail=False):
                ops = []
                for tt in range(4 * s, 4 * (s + 1)):
                    st = {}

                    def op_ec(ec, tt=tt, st=st):
                        def go():
                            ps = ps_fill.tile([128, 512], F32, name="opj", tag="fill")
                            st[ec] = ps
                            for i in range(2):
                                nc.tensor.matmul(
                                    ps[:],
                                    lhsT=attnT[
                                        :, i * T + tt * 128 : i * T + (tt + 1) * 128
                                    ],
                                    rhs=wo_bf[
                                        :, i * D + ec * 512 : i * D + (ec + 1) * 512
                                    ],
                                    start=(i == 0),
                                    stop=(i == 1),
                                )

                        return go

                    def cast_dma(ec, tt=tt, st=st):
                        def go():
                            if ec == 0:
                                st["osb"] = osb_pool.tile(
                                    [128, 1024], BF16, name="osb"
                                )
                            if tail and ec == 0:
                                nc.scalar.copy(
                                    st["osb"][:, 0:512], st[ec][:]
                                )
                            else:
                                nc.vector.tensor_copy(
                                    st["osb"][:, ec * 512 : (ec + 1) * 512], st[ec][:]
                                )
                            if tail:
                                eng = nc.sync if ec == 0 else nc.scalar
                                eng.dma_start(
                                    out=out[
                                        tt * 128 : (tt + 1) * 128,
                                        ec * 512 : (ec + 1) * 512,
                                    ],
                                    in_=st["osb"][:, ec * 512 : (ec + 1) * 512],
                                )
                            elif ec == 1:
                                eng = nc.sync if tt % 2 == 0 else nc.gpsimd
                                eng.dma_start(
                                    out=out[tt * 128 : (tt + 1) * 128, :],
                                    in_=st["osb"][:],
                                )

                        return go

                    ops.extend([op_ec(0), op_ec(1), cast_dma(0), cast_dma(1)])
                return ops

            def interleave(a, b):
                if not a:
                    return list(b)
                if not b:
                    return list(a)
                res = []
                nb, na, bi = len(b), len(a), 0
                for i, op in enumerate(a):
                    res.append(op)
                    want = (i + 1) * nb // na
                    while bi < want:
                        res.append(b[bi])
                        bi += 1
                res.extend(b[bi:])
                return res

            # ---- prologue: wave 0 (QK chunk 0 + V tiles 0-3) ----
            for op in qk_thunks(0):
                op()
            for op in v_thunks(range(0, 4)):
                op()

            waves = {
                0: qk_thunks(1) + v_thunks(range(4, 8)),
                1: qk_thunks(2) + v_thunks(range(8, 12)),
                2: qk_thunks(3) + v_thunks(range(12, 16)),
            }

            HORD = [1, 3, 0, 2]  # odd heads first: their attnT DMA-shifts
            # happen early; the final pair of each slab writes attnT direct
            pairs = [(s, h) for s in range(NSLAB) for h in HORD]
            pts = {}
            refs = {}
            prev = None
            pending_epi = []  # list of per-idx op lists
            for idx in range(len(pairs)):
                s, h = pairs[idx]
                pts[idx] = pt_pool.tile([128, TT * 512], BF16, name="pt")
                sc = scores_chunks(s, h, pts[idx])
                blist = []
                if prev is not None:
                    ps_, ph_ = pairs[prev]
                    refs[prev] = {}
                    av_a, av_b = av_ops_ab(ps_, ph_, pts[prev], refs[prev])
                    nrm = norm_ops(ps_, ph_, refs[prev], 0, 512)
                    epi = list(pending_epi.pop(0)) if pending_epi else []
                    if idx % 4 == 1 and s >= 1:
                        eall = epilogue_ops(s - 1)
                        epi += eall[:4]
                        pending_epi = [eall[4:8], eall[8:12], eall[12:16]]
                    fill = waves[s][2 * (idx % 4) : 2 * (idx % 4) + 2] if s < 3 else []
                    blist = (
                        [av_a]
                        + epi[:6]
                        + fill
                        + [av_b]
                        + nrm
                        + epi[6:]
                    )
                else:
                    fill = waves[0][0:2]
                    blist = list(fill)
                for op in interleave(sc, blist):
                    op()
                prev = idx

            # ---- final pair (3,3): split AV/norm halves interleaved with
            # the slab-3 epilogue ----
            s_, h_ = 3, 2
            ref = {}
            base, off = pt_layout(s_)
            nk = 16
            pt = pts[15]

            def av15a():
                avb = ps_av.tile([128, 512], F32, name="psav", tag="psav")
                ref["avb"] = avb
                kts = [kt for kt in range(nk) if off[kt] < 256]
                for i_, kt in enumerate(kts):
                    o = off[kt]
                    nc.tensor.matmul(
                        avb[0:65, o:256],
                        lhsT=vb4[:, kt * NH + h_, :],
                        rhs=pt[:, base[kt] : base[kt] + 256 - o],
                        start=(i_ == 0),
                        stop=(i_ == len(kts) - 1),
                    )

            def av15b():
                avb = ref["avb"]
                for kt in range(nk):
                    o2 = max(off[kt] - 256, 0)
                    c0 = base[kt] + 256 - off[kt] + o2
                    nc.tensor.matmul(
                        avb[0:65, 256 + o2 : 512],
                        lhsT=vb4[:, kt * NH + h_, :],
                        rhs=pt[:, c0 : c0 + 256 - o2],
                        start=(kt == 0),
                        stop=(kt == nk - 1),
                    )

            nrm_a = norm_ops(s_, h_, ref, 0, 256)
            nrm_b = norm_ops(s_, h_, ref, 256, 512)
            eops = epilogue_ops(3, tail=True)

            for lst in pending_epi:  # EP(2) remnant: exp-free PE filler
                for op_ in lst:
                    op_()
            av15a()
            for op_ in nrm_a:
                op_()
            for op_ in eops[0:4]:  # tt12
                op_()
            av15b()
            for op_ in eops[4:8]:  # tt13
                op_()
            for op_ in nrm_b:
                op_()
            for op_ in eops[8:16]:  # tt14, tt15
                op_()

    nc.compile()
    return nc


def _get_nc():
    global _NC_CACHE
    if _NC_CACHE is None:
        _NC_CACHE = build()
    return _NC_CACHE


def make_in_maps(x, Wq, Wk, Wv, Wo):
    bf = ml_dtypes.bfloat16
    x = np.asarray(x, dtype=np.float32)
    WqT = np.asarray(Wq, dtype=np.float32).astype(bf)
    WkT = np.asarray(Wk, dtype=np.float32).astype(bf)
    WvT = np.asarray(Wv, dtype=np.float32).astype(bf)
    WoT = np.asarray(Wo, dtype=np.float32).astype(bf)

    def x_img(xb):  # [1024(d), 2048(t)] -> [128, c*4096 + dt*512 + j]
        return np.ascontiguousarray(
            xb.reshape(DT, 128, NSLAB, 512).transpose(1, 2, 0, 3).reshape(128, -1)
        )

    def qk_img(w):  # [1024, 256] -> m-major [128, m*1024 + dt*128 + c]
        return np.ascontiguousarray(
            w.reshape(DT, 128, 2, 128).transpose(1, 2, 0, 3).reshape(128, -1)
        )

    def v_img(w):  # [1024, 256] -> dt-major [128, dt*256 + c]
        return np.ascontiguousarray(
            w.reshape(DT, 128, DH).transpose(1, 0, 2).reshape(128, -1)
        )

    def o_img(w):  # [256, 1024] -> i-major [128, i*1024 + c]
        return np.ascontiguousarray(
            w.reshape(2, 128, D).transpose(1, 0, 2).reshape(128, -1)
        )

    xTb = [x_img(x[b].T.astype(bf)) for b in range(2)]
    in_maps = []
    for core in range(8):
        b, g = core // 4, core % 4
        sl = slice(g * DH, (g + 1) * DH)
        in_maps.append(
            {
                "xT": xTb[b],
                "Wq": qk_img(WqT[:, sl]),
                "Wk": qk_img(WkT[:, sl]),
                "Wv": v_img(WvT[:, sl]),
                "Wo": o_img(WoT[sl, :]),
            }
        )
    return in_maps


def unshard(results):
    out = np.empty((2, T, D), np.float32)
    for b in range(2):
        acc = results[4 * b]["out"].astype(np.float32)
        for g in range(1, 4):
            acc = acc + results[4 * b + g]["out"].astype(np.float32)
        out[b] = acc
    return out


def kernel(x, Wq, Wk, Wv, Wo):
    nc = _get_nc()
    in_maps = make_in_maps(x, Wq, Wk, Wv, Wo)
    res = run_bass_kernel_spmd(nc, in_maps, core_ids=list(range(8)))
    return unshard(res.results)


# revision 25
# speedup vs baseline: 1.0129x; 1.0129x over previous
"""Distributed causal multi-head attention for 8 TRN2 NeuronCores (v2).

Problem: B=2, T=2048, D=1024, H=16 heads (hd=64), f32 in/out.

Sharding: core i handles batch b=i//4 and head-group g=i%4 (4 heads).
Wq/Wk/Wv column-sharded ([1024, 256] per core), Wo row-sharded
([256, 1024] per core).  Each core computes a partial output projection
for its 4 heads over the full sequence; the host sums the 4 partials
per batch (the unshard step replaces the all-reduce).  Weights and
activations are pre-cast to bf16 on the host; x is laid out transposed
(xT = x^T).  Output partials are shipped bf16 and summed f32 on host.

v2 dataflow changes vs v1:
  - softmax normalize applied directly on the AV output (O^T layout,
    dh on partitions) via: den-row cast -> K=1 ones-matmul broadcast
    (den replicated over 64 partitions in PSUM) -> reciprocal_approx
    -> one tensor_tensor multiply writing attnT in place.  This deletes
    all 96 PE transposes of v1 (4 per pair + 2 per q-tile).
  - odd heads (attnT partitions 64-127) land via a cheap SBUF->SBUF
    DMA partition shift (DVE lanes are partition-locked).
  - QKV projections stream as 512-column wavefront thunks used as PE
    filler: wave c feeds q-slab c, emitted one slab ahead, so attention
    starts ~3us into the kernel and the PE never starves on input DMA.
  - out projection results are cast bf16 and DMA'd per q-tile (halves
    output traffic; host sums partials in f32).
  - exp table-load is prefetched with a dummy activation at t=0.
"""

import numpy as np
import ml_dtypes

import concourse.bass as bass
import concourse.mybir as mybir
import concourse.tile as tile
from concourse import bacc
from concourse.bass_utils import run_bass_kernel_spmd

F32 = mybir.dt.float32
BF16 = mybir.dt.bfloat16
AF = mybir.ActivationFunctionType
MULT = mybir.AluOpType.mult

T = 2048  # sequence length
D = 1024  # embed dim
NH = 4  # heads per core
HD = 64  # head dim
DH = NH * HD  # 256, sharded d per core
TT = T // 128  # 16 t tiles
DT = D // 128  # 8 embed tiles
NSLAB = 4  # q slabs of 512
SCALE = 1.0 / np.sqrt(HD)

_NC_CACHE = None


def build():
    nc = bacc.Bacc(None, target_bir_lowering=False, debug=False)

    # inputs are shipped as ready-to-DMA SBUF images (see make_in_maps):
    # xT_img[p, c*4096 + dt*512 + j] = x[c*512+j, dt*128+p]  (chunk-major)
    # wq/wk m-major [p, m*1024 + dt*128 + c]; wv dt-major [p, dt*256 + c];
    # wo i-major [p, i*1024 + c]
    xT_img = nc.declare_dram_parameter("xT", [128, NSLAB * DT * 512], BF16, isOutput=False)
    wq = nc.declare_dram_parameter("Wq", [128, 2 * DT * 128], BF16, isOutput=False)
    wk = nc.declare_dram_parameter("Wk", [128, 2 * DT * 128], BF16, isOutput=False)
    wv = nc.declare_dram_parameter("Wv", [128, DT * DH], BF16, isOutput=False)
    wo = nc.declare_dram_parameter("Wo", [128, 2 * D], BF16, isOutput=False)
    out = nc.declare_dram_parameter("out", [T, D], BF16, isOutput=True)

    with tile.TileContext(nc) as tc:
        with (
            tc.tile_pool(name="persist", bufs=1) as persist,
            tc.tile_pool(name="pt", bufs=2) as pt_pool,
            tc.tile_pool(name="den", bufs=2) as den_pool,
            tc.tile_pool(name="rc", bufs=2) as rc_pool,
            tc.tile_pool(name="stg", bufs=2) as stg_pool,
            tc.tile_pool(name="osb", bufs=2) as osb_pool,
            tc.tile_pool(name="ps_big", bufs=2, space="PSUM") as ps_big,
            tc.tile_pool(name="ps_fill", bufs=2, space="PSUM") as ps_fill,
            tc.tile_pool(name="ps_av", bufs=2, space="PSUM") as ps_av,
        ):
            def P(shape, dtype, name):
                return persist.tile(shape, dtype, name=name, tag=name)

            ones_b = P([128, 64], BF16, "ones_b")
            junk = P([128, 16], F32, "junk")
            jout = P([128, 16], F32, "jout")
            jnk_b = P([128, 512], BF16, "jnk_b")

            wq_bf = P([128, DT * DH], BF16, "wq_bf")
            wk_bf = P([128, DT * DH], BF16, "wk_bf")
            wv_bf = P([128, DT * DH], BF16, "wv_bf")
            wo_bf = P([128, 2 * D], BF16, "wo_bf")
            xT = P([128, DT * T], BF16, "xT")
            QT = P([128, 2 * T], BF16, "QT")
            KT = P([128, 2 * T], BF16, "KT")
            vbuf = P([128, TT * NH * 65], BF16, "vbuf")
            attnT = P([128, 2 * T], BF16, "attnT")

            # memsets first (gpsimd, ~0.4us) so the HAM warm-up dummies
            # can start at ~7us while the input DMAs land
            nc.gpsimd.memset(ones_b[:], 1.0)
            nc.gpsimd.memset(junk[:], 0.0)
            nc.gpsimd.memset(jnk_b[0:1, :], 1.0)
            # exp table prefetch: overlaps the ~2.7us ACT_TABLE_LOAD with
            # the input DMAs instead of paying it at the first real score
            nc.scalar.activation(out=jout[:], in_=junk[:], func=AF.Exp, scale=1.0)
            # ---- input DMAs: issued first, pre-arranged images, 8-32KB
            # contiguous lines, split across sync/gpsimd/scalar queues ----
            xT3 = xT.rearrange("p (dt t) -> p dt t", dt=DT)

            def xi3(c, d0, d1):
                return xT_img[:, c * 4096 + d0 * 512 : c * 4096 + d1 * 512].rearrange(
                    "p (dt t) -> p dt t", dt=d1 - d0
                )

            def xc_dma(eng, c, d0, d1):
                eng.dma_start(
                    out=xT3[:, d0:d1, c * 512 : (c + 1) * 512], in_=xi3(c, d0, d1)
                )

            # first wave = exactly what the first thunks need (wq_m0 +
            # xT chunk 0, 1.25MB): the three rings share the 16 DMA
            # engines, so nothing else competes until these are in flight
            nc.sync.dma_start(out=wq_bf[:, 0:1024], in_=wq[:, 0:1024])
            xc_dma(nc.scalar, 0, 0, 4)
            xc_dma(nc.gpsimd, 0, 4, 8)
            nc.sync.dma_start(out=wq_bf[:, 1024:2048], in_=wq[:, 1024:2048])
            nc.scalar.dma_start(out=wk_bf[:, 0:1024], in_=wk[:, 0:1024])
            nc.gpsimd.dma_start(out=wk_bf[:, 1024:2048], in_=wk[:, 1024:2048])
            nc.sync.dma_start(out=wv_bf[:], in_=wv[:])
            xc_dma(nc.scalar, 1, 0, 4)
            xc_dma(nc.gpsimd, 1, 4, 8)
            nc.sync.dma_start(out=wo_bf[:], in_=wo[:])
            xc_dma(nc.scalar, 2, 0, 4)
            xc_dma(nc.gpsimd, 2, 4, 8)
            xc_dma(nc.sync, 3, 0, 4)
            xc_dma(nc.gpsimd, 3, 4, 8)

            # HAM warm-up: ~8 dummy matmuls (~4us cold) run during the
            # DMA wait so the first real matmuls start near 2.4GHz
            warm_ps = ps_fill.tile([128, 512], F32, name="warm", tag="fill")
            for _ in range(8):
                nc.tensor.matmul(
                    warm_ps[0:64, 0:512],
                    lhsT=ones_b[0:1, 0:64],
                    rhs=jnk_b[0:1, 0:512],
                    start=True,
                    stop=True,
                )
            vb3 = vbuf.rearrange("p (t c) -> p t c", c=65)
            nc.gpsimd.memset(vb3[:, :, 64:65], 1.0)
            vb4 = vbuf.rearrange("p (n c) -> p n c", c=65)

            # ---- projection wavefront thunks (PE filler) ----
            def qk_thunks(c):
                th = []
                for w_bf, outT in ((wq_bf, QT), (wk_bf, KT)):
                    for m in range(2):
                        def go(w_bf=w_bf, outT=outT, m=m, c=c):
                            ps = ps_fill.tile([128, 512], F32, name="qk", tag="fill")
                            for dt_ in range(DT):
                                nc.tensor.matmul(
                                    ps[:],
                                    lhsT=w_bf[
                                        :,
                                        m * 1024 + dt_ * 128 : m * 1024 + (dt_ + 1) * 128,
                                    ],
                                    rhs=xT[
                                        :, dt_ * T + c * 512 : dt_ * T + (c + 1) * 512
                                    ],
                                    start=(dt_ == 0),
                                    stop=(dt_ == DT - 1),
                                )
                            nc.vector.tensor_copy(
                                outT[:, m * T + c * 512 : m * T + (c + 1) * 512],
                                ps[:],
                            )

                        th.append(go)
                return th

            def v_thunks(tts):
                th = []
                for tt in tts:
                    def go(tt=tt):
                        ps = ps_fill.tile([128, 256], F32, name="vp", tag="fill")
                        for dt_ in range(DT):
                            nc.tensor.matmul(
                                ps[:],
                                lhsT=xT[
                                    :, dt_ * T + tt * 128 : dt_ * T + (tt + 1) * 128
                                ],
                                rhs=wv_bf[:, dt_ * DH : (dt_ + 1) * DH],
                                start=(dt_ == 0),
                                stop=(dt_ == DT - 1),
                            )
                        nc.vector.tensor_copy(
                            vb4[:, tt * NH : (tt + 1) * NH, 0:64],
                            ps.rearrange("p (n c) -> p n c", n=NH),
                        )

                    th.append(go)
                return th

            # ---- scores ----
            def pt_layout(s):
                """Compact per-pair PT layout: col base and q-offset per kt."""
                base, off, b = {}, {}, 0
                for kt in range(4 * (s + 1)):
                    j = kt - 4 * s
                    o = 128 * j if j > 0 else 0
                    base[kt], off[kt] = b, o
                    b += 512 - o
                return base, off

            def scores_chunks(s, h, pt):
                m, r0 = h // 2, (h % 2) * 64
                base, _ = pt_layout(s)

                def off_diag(kt):
                    def go():
                        ps = ps_big.tile([128, 1024], F32, name="psst")
                        for u in range(2):
                            nc.tensor.matmul(
                                ps[:, u * 512 : (u + 1) * 512],
                                lhsT=KT[
                                    r0 : r0 + 64,
                                    m * T + (kt + u) * 128 : m * T + (kt + u + 1) * 128,
                                ],
                                rhs=QT[
                                    r0 : r0 + 64,
                                    m * T + s * 512 : m * T + (s + 1) * 512,
                                ],
                                start=True,
                                stop=True,
                            )
                        nc.scalar.activation(
                            out=pt[:, base[kt] : base[kt] + 1024],
                            in_=ps[:],
                            func=AF.Exp,
                            scale=float(SCALE),
                        )

                    return go

                def diag2(j0):
                    widths = [512 - 128 * j0, 512 - 128 * (j0 + 1)]
                    wtot = sum(widths)

                    def go():
                        ps = ps_big.tile([128, 1024], F32, name="psst")
                        o = 0
                        for u, w in enumerate(widths):
                            j = j0 + u
                            kt = 4 * s + j
                            nc.tensor.matmul(
                                ps[:, o : o + w],
                                lhsT=KT[
                                    r0 : r0 + 64,
                                    m * T + kt * 128 : m * T + (kt + 1) * 128,
                                ],
                                rhs=QT[
                                    r0 : r0 + 64,
                                    m * T + s * 512 + 128 * j : m * T + (s + 1) * 512,
                                ],
                                start=True,
                                stop=True,
                            )
                            o += w
                        kt0 = 4 * s + j0
                        nc.scalar.activation(
                            out=pt[:, base[kt0] : base[kt0] + wtot],
                            in_=ps[:, 0:wtot],
                            func=AF.Exp,
                            scale=float(SCALE),
                        )
                        for u in range(2):
                            kt = 4 * s + j0 + u
                            nc.gpsimd.affine_select(
                                out=pt[:, base[kt] : base[kt] + 128],
                                in_=pt[:, base[kt] : base[kt] + 128],
                                pattern=[[1, 128]],
                                compare_op=mybir.AluOpType.is_ge,
                                fill=0.0,
                                base=0,
                                channel_multiplier=-1,
                            )

                    return go

                return [off_diag(2 * u) for u in range(2 * s)] + [diag2(0), diag2(2)]

            # ---- AV + normalize ----
            def av_ops_ab(s, h, pt, ref):
                """AV accumulation split: A = off-diag k-tiles (exp long
                done), B = the 4 diagonal k-tiles (gated on last exps)."""
                base, off = pt_layout(s)
                nk = 4 * (s + 1)

                def av_a():
                    avb = ps_av.tile([128, 512], F32, name="psav", tag="psav")
                    ref["avb"] = avb
                    for kt in range(4 * s):
                        nc.tensor.matmul(
                            avb[0:65, 0:512],
                            lhsT=vb4[:, kt * NH + h, :],
                            rhs=pt[:, base[kt] : base[kt] + 512],
                            start=(kt == 0),
                            stop=False,
                        )

                def av_b():
                    avb = ref["avb"]
                    for kt in range(4 * s, nk):
                        o = off[kt]
                        nc.tensor.matmul(
                            avb[0:65, o:512],
                            lhsT=vb4[:, kt * NH + h, :],
                            rhs=pt[:, base[kt] : base[kt] + 512 - o],
                            start=(kt == 0),
                            stop=(kt == nk - 1),
                        )

                return av_a, av_b

            def norm_ops(s, h, ref, q0, q1):
                """den row -> bf16 -> K=1 ones-matmul broadcast -> recip
                -> tensor_tensor multiply -> attnT (DMA shift for odd h)."""
                i_c = h // 2
                c0 = i_c * T + s * 512
                odd = h % 2 == 1
                st = {}

                def d1():
                    den = den_pool.tile([128, 512], BF16, name="den")
                    st["den"] = den
                    nc.vector.tensor_copy(
                        den[64:65, q0:q1], ref["avb"][64:65, q0:q1]
                    )

                def m1():
                    denb = ps_fill.tile([128, 512], F32, name="denb", tag="fill")
                    st["denb"] = denb
                    nc.tensor.matmul(
                        denb[0:64, q0:q1],
                        lhsT=ones_b[64:65, 0:64],
                        rhs=st["den"][64:65, q0:q1],
                        start=True,
                        stop=True,
                    )

                def d2():
                    rc = rc_pool.tile([128, 512], F32, name="rc")
                    st["rc"] = rc
                    nc.vector.reciprocal_approx_fast(
                        rc[0:64, q0:q1], st["denb"][0:64, q0:q1]
                    )

                def d3():
                    if odd:
                        stg = stg_pool.tile([128, 512], BF16, name="stg")
                        st["stg"] = stg
                        dst = stg[0:64, q0:q1]
                    else:
                        dst = attnT[0:64, c0 + q0 : c0 + q1]
                    nc.vector.tensor_tensor(
                        out=dst,
                        in0=ref["avb"][0:64, q0:q1],
                        in1=st["rc"][0:64, q0:q1],
                        op=MULT,
                    )

                def d4():
                    nc.gpsimd.dma_start(
                        out=attnT[64:128, c0 + q0 : c0 + q1],
                        in_=st["stg"][0:64, q0:q1],
                    )

                ops = [d1, m1, d2, d3]
                if odd:
                    ops.append(d4)
                return ops

            # ---- out projection epilogue.  tail=True (final slab): ec0
            # cast moves to the now-idle scalar engine and each 512-col
            # half DMAs out as soon as its cast lands ----
            def epilogue_ops(s, tail=False):
                ops = []
                for tt in range(4 * s, 4 * (s + 1)):
                    st = {}

                    def op_ec(ec, tt=tt, st=st):
                        def go():
                            ps = ps_fill.tile([128, 512], F32, name="opj", tag="fill")
                            st[ec] = ps
                            for i in range(2):
                                nc.tensor.matmul(
                                    ps[:],
                                    lhsT=attnT[
                                        :, i * T + tt * 128 : i * T + (tt + 1) * 128
                                    ],
                                    rhs=wo_bf[
                                        :, i * D + ec * 512 : i * D + (ec + 1) * 512
                                    ],
                                    start=(i == 0),
                                    stop=(i == 1),
                                )

                        return go

                    def cast_dma(ec, tt=tt, st=st):
                        def go():
                            if ec == 0:
                                st["osb"] = osb_pool.tile(
                                    [128, 1024], BF16, name="osb"
                                )
                            if tail and ec == 0:
                                nc.scalar.copy(
                                    st["osb"][:, 0:512], st[ec][:]
                                )
                            else:
                                nc.vector.tensor_copy(
                                    st["osb"][:, ec * 512 : (ec + 1) * 512], st[ec][:]
                                )
                            if tail:
                                eng = nc.sync if ec == 0 else nc.scalar
                                eng.dma_start(
                                    out=out[
                                        tt * 128 : (tt + 1) * 128,
                                        ec * 512 : (ec + 1) * 512,
                                    ],
                                    in_=st["osb"][:, ec * 512 : (ec + 1) * 512],
                                )
                            elif ec == 1:
                                eng = nc.sync if tt % 2 == 0 else nc.gpsimd
                                eng.dma_start(
                                    out=out[tt * 128 : (tt + 1) * 128, :],
                                    in_=st["osb"][:],
                                )

                        return go

                    ops.extend([op_ec(0), op_ec(1), cast_dma(0), cast_dma(1)])
                return ops

            def interleave(a, b):
                if not a:
                    return list(b)
                if not b:
                    return list(a)
                res = []
                nb, na, bi = len(b), len(a), 0
                for i, op in enumerate(a):
                    res.append(op)
                    want = (i + 1) * nb // na
                    while bi < want:
                        res.append(b[bi])
                        bi += 1
                res.extend(b[bi:])
                return res

            # ---- prologue: wave 0 (QK chunk 0 + V tiles 0-3) ----
            for op in qk_thunks(0):
                op()
            for op in v_thunks(range(0, 4)):
                op()

            waves = {
                0: qk_thunks(1) + v_thunks(range(4, 8)),
                1: qk_thunks(2) + v_thunks(range(8, 12)),
                2: qk_thunks(3) + v_thunks(range(12, 16)),
            }

            HORD = [1, 3, 0, 2]  # odd heads first: their attnT DMA-shifts
            # happen early; the final pair of each slab writes attnT direct
            pairs = [(s, h) for s in range(NSLAB) for h in HORD]
            pts = {}
            refs = {}
            prev = None
            pending_epi = []  # list of per-idx op lists
            for idx in range(len(pairs)):
                s, h = pairs[idx]
                pts[idx] = pt_pool.tile([128, TT * 512], BF16, name="pt")
                sc = scores_chunks(s, h, pts[idx])
                blist = []
                if prev is not None:
                    ps_, ph_ = pairs[prev]
                    refs[prev] = {}
                    av_a, av_b = av_ops_ab(ps_, ph_, pts[prev], refs[prev])
                    nrm = norm_ops(ps_, ph_, refs[prev], 0, 512)
                    epi = list(pending_epi.pop(0)) if pending_epi else []
                    if idx % 4 == 1 and s >= 1:
                        eall = epilogue_ops(s - 1)
                        epi += eall[:4]
                        pending_epi = [eall[4:8], eall[8:12], eall[12:16]]
                    fill = waves[s][2 * (idx % 4) : 2 * (idx % 4) + 2] if s < 3 else []
                    blist = (
                        [av_a]
                        + epi[:6]
                        + fill
                        + [av_b]
                        + nrm
                        + epi[6:]
                    )
                else:
                    fill = waves[0][0:2]
                    blist = list(fill)
                for op in interleave(sc, blist):
                    op()
                prev = idx

            # ---- final pair (3,3): split AV/norm halves interleaved with
            # the slab-3 epilogue ----
            s_, h_ = 3, 2
            ref = {}
            base, off = pt_layout(s_)
            nk = 16
            pt = pts[15]

            def av15a():
                avb = ps_av.tile([128, 512], F32, name="psav", tag="psav")
                ref["avb"] = avb
                kts = [kt for kt in range(nk) if off[kt] < 256]
                for i_, kt in enumerate(kts):
                    o = off[kt]
                    nc.tensor.matmul(
                        avb[0:65, o:256],
                        lhsT=vb4[:, kt * NH + h_, :],
                        rhs=pt[:, base[kt] : base[kt] + 256 - o],
                        start=(i_ == 0),
                        stop=(i_ == len(kts) - 1),
                    )

            def av15b():
                avb = ref["avb"]
                for kt in range(nk):
                    o2 = max(off[kt] - 256, 0)
                    c0 = base[kt] + 256 - off[kt] + o2
                    nc.tensor.matmul(
                        avb[0:65, 256 + o2 : 512],
                        lhsT=vb4[:, kt * NH + h_, :],
                        rhs=pt[:, c0 : c0 + 256 - o2],
                        start=(kt == 0),
                        stop=(kt == nk - 1),
                    )

            nrm_a = norm_ops(s_, h_, ref, 0, 256)
            # last norm split in two 128-col chains: both PE-broadcasts
            # issue up front, tt14's matmuls start as soon as the first
            # chain lands while the second chain drains on the DVE
            nrm_b1 = norm_ops(s_, h_, ref, 256, 384)
            nrm_b2 = norm_ops(s_, h_, ref, 384, 512)
            eops = epilogue_ops(3, tail=True)

            for lst in pending_epi:  # EP(2) remnant: exp-free PE filler
                for op_ in lst:
                    op_()
            av15a()
            for op_ in nrm_a:
                op_()
            for op_ in eops[0:4]:  # tt12
                op_()
            av15b()
            for op_ in eops[4:8]:  # tt13
                op_()
            nrm_b1[0]()  # den casts + broadcasts first
            nrm_b1[1]()
            nrm_b2[0]()
            nrm_b2[1]()
            nrm_b1[2]()  # recip + multiply chains
            nrm_b1[3]()
            nrm_b2[2]()
            nrm_b2[3]()
            for op_ in eops[8:12]:  # tt14 (gated on chain b1)
                op_()
            for op_ in eops[12:16]:  # tt15 (gated on chain b2)
                op_()

    nc.compile()
    return nc


def _get_nc():
    global _NC_CACHE
    if _NC_CACHE is None:
        _NC_CACHE = build()
    return _NC_CACHE


def make_in_maps(x, Wq, Wk, Wv, Wo):
    bf = ml_dtypes.bfloat16
    x = np.asarray(x, dtype=np.float32)
    WqT = np.asarray(Wq, dtype=np.float32).astype(bf)
    WkT = np.asarray(Wk, dtype=np.float32).astype(bf)
    WvT = np.asarray(Wv, dtype=np.float32).astype(bf)
    WoT = np.asarray(Wo, dtype=np.float32).astype(bf)

    def x_img(xb):  # [1024(d), 2048(t)] -> [128, c*4096 + dt*512 + j]
        return np.ascontiguousarray(
            xb.reshape(DT, 128, NSLAB, 512).transpose(1, 2, 0, 3).reshape(128, -1)
        )

    def qk_img(w):  # [1024, 256] -> m-major [128, m*1024 + dt*128 + c]
        return np.ascontiguousarray(
            w.reshape(DT, 128, 2, 128).transpose(1, 2, 0, 3).reshape(128, -1)
        )

    def v_img(w):  # [1024, 256] -> dt-major [128, dt*256 + c]
        return np.ascontiguousarray(
            w.reshape(DT, 128, DH).transpose(1, 0, 2).reshape(128, -1)
        )

    def o_img(w):  # [256, 1024] -> i-major [128, i*1024 + c]
        return np.ascontiguousarray(
            w.reshape(2, 128, D).transpose(1, 0, 2).reshape(128, -1)
        )

    xTb = [x_img(x[b].T.astype(bf)) for b in range(2)]
    in_maps = []
    for core in range(8):
        b, g = core // 4, core % 4
        sl = slice(g * DH, (g + 1) * DH)
        in_maps.append(
            {
                "xT": xTb[b],
                "Wq": qk_img(WqT[:, sl]),
                "Wk": qk_img(WkT[:, sl]),
                "Wv": v_img(WvT[:, sl]),
                "Wo": o_img(WoT[sl, :]),
            }
        )
    return in_maps


def unshard(results):
    out = np.empty((2, T, D), np.float32)
    for b in range(2):
        acc = results[4 * b]["out"].astype(np.float32)
        for g in range(1, 4):
            acc = acc + results[4 * b + g]["out"].astype(np.float32)
        out[b] = acc
    return out


def kernel(x, Wq, Wk, Wv, Wo):
    nc = _get_nc()
    in_maps = make_in_maps(x, Wq, Wk, Wv, Wo)
    res = run_bass_kernel_spmd(nc, in_maps, core_ids=list(range(8)))
    return unshard(res.results)
